# revision 11
# baseline (speedup 1.0000x reference)
"""Trainium2 Bass kernel for nn_CT_Attention (conv-transformer cross-attention).

Sharding: data-parallel over batch (B=8) across 8 NeuronCores; one image pair
per core, weights replicated. No collectives.

Per-core pipeline (matmul operands bf16, accumulation fp32 in PSUM):
  host pre-pads x/y (bf16, width padded to 36 for DVE alignment) -> depthwise
  3x3 conv: taps split across PE (diagonal matmuls, PSUM accumulate), DVE and
  GPSIMD (fused multiply-accumulate), combined at drain -> pointwise convs as
  matmuls -> cross-attention with transposed scores (S_T = k^T q) so the
  softmax key-reduction runs as ones-matmuls whose M=64 output replicates the
  row-sum across partitions (normalization then needs no partition broadcast);
  V is produced directly transposed by the KV pointwise matmul -> PV with
  column-packed head pairs -> DVE normalization -> output projection + bias ->
  DMA out.  Accumulation groups sharing a PSUM bank are kept sequential
  (start=True clears has_written for the whole bank).

BN folding (host): scale into dw tap weights; bias into the q-path ACT bias
(k's bias shifts scores per-query only -> softmax invariant -> dropped; v's
bias times row-stochastic attention = constant -> folded into out-proj bias).
"""

import os
import sys

import numpy as np

for _p in ("/root/.axon_site", "/root/.axon_site/_ro/trn_rl_repo",
           "/root/.axon_site/_ro/pypackages", "/opt/trn_rl_repo", "/opt/pypackages"):
    if os.path.isdir(_p) and _p not in sys.path:
        sys.path.append(_p)

import ml_dtypes

BF16 = ml_dtypes.bfloat16

DIM = 256
HEADS = 8
DHEAD = 64
INNER = 512
SCALE = DHEAD ** -0.5
EPS = 1e-5
H = W = 32
HWN = H * W          # 1024 query positions
J = 256              # 16*16 key positions
NCORES = 8

# depthwise tap split by engine; DVE taps need dx = +-1 (t%3 != 1) so the
# bf16 2x_1P alignment (4B) holds on the width-36 padded rows.
GP_TAPS = (0, 2)
DVE_TAPS = (8, 5)
PE_TAPS = tuple(t for t in range(9) if t not in GP_TAPS and t not in DVE_TAPS)

_STATE = {}


# ----------------------------------------------------------------------------
# host-side preprocessing
# ----------------------------------------------------------------------------

def _prep_weights(inp):
    f = np.float32
    s1 = (inp["q_bn_gamma"] / np.sqrt(inp["q_bn_var"] + f(EPS))).astype(f)
    b1 = (inp["q_bn_beta"] - inp["q_bn_mean"] * s1).astype(f)
    s2 = (inp["kv_bn_gamma"] / np.sqrt(inp["kv_bn_var"] + f(EPS))).astype(f)
    b2 = (inp["kv_bn_beta"] - inp["kv_bn_mean"] * s2).astype(f)

    def diag_taps(dw_w, s, taps):
        # [128, len(taps), 2, 128] partition-major, bf16, single DMA
        d = np.zeros((128, len(taps), 2, 128), f)
        idx = np.arange(128)
        for i, t in enumerate(taps):
            wt = dw_w[:, 0, t // 3, t % 3] * s          # [256]
            for kc in range(2):
                d[idx, i, kc, idx] = wt[kc * 128:(kc + 1) * 128]
        return d.astype(BF16)

    def tapw(dw_w, s):
        # per-partition tap scalars [128, 9, 2] fp32
        w = (dw_w[:, 0].reshape(256, 9) * s[:, None]).astype(f)    # [256, 9]
        return np.ascontiguousarray(
            w.reshape(2, 128, 9).transpose(1, 2, 0))               # [128, 9, 2]

    dq = diag_taps(inp["q_dw_w"], s1, PE_TAPS)
    dkv = diag_taps(inp["kv_dw_w"], s2, tuple(range(9)))
    tq = tapw(inp["q_dw_w"], s1)

    qpw = inp["q_pw_w"].astype(f)                        # [512, 256]
    wq = np.ascontiguousarray(
        (f(SCALE) * qpw).T.reshape(2, 128, 512).transpose(1, 0, 2)).astype(BF16)
    qb = (f(SCALE) * (qpw @ b1)).reshape(4, 128).T.copy()  # [128, 4] fp32

    kvpw = inp["kv_pw_w"].astype(f)                      # [1024, 256]
    wk = np.ascontiguousarray(
        kvpw[:512].T.reshape(2, 128, 512).transpose(1, 0, 2)).astype(BF16)
    wvT = np.ascontiguousarray(
        kvpw[512:].T.reshape(2, 128, 512).transpose(1, 0, 2)).astype(BF16)
    vb = kvpw[512:] @ b2                                 # [512]

    ow = inp["out_w"].astype(f)                          # [256, 512]
    wo = np.ascontiguousarray(
        ow.T.reshape(4, 128, 256).transpose(1, 0, 2)).astype(BF16)
    ob = (inp["out_b"].astype(f) + ow @ vb).reshape(2, 128).T.copy()  # [128, 2]

    return dict(dq=dq, dkv=dkv, tq=tq, wq=wq, wk=wk, wvT=wvT, wo=wo,
                qb=qb, ob=ob)


def _pad_image(img):
    # [256, 32, 32] -> [128, 2, 34, 36] zero-padded bf16, partition-major
    out = np.zeros((128, 2, 34, 36), BF16)
    out[:, :, 1:33, 1:33] = img.reshape(2, 128, 32, 32).transpose(1, 0, 2, 3)
    return out


# ----------------------------------------------------------------------------
# device program
# ----------------------------------------------------------------------------

def _emit(ctx, tc, outs, ins):
    import concourse.bass as bass  # noqa: F401
    from concourse import mybir

    nc = tc.nc
    f32 = mybir.dt.float32
    bf16 = mybir.dt.bfloat16
    mult = mybir.AluOpType.mult
    add = mybir.AluOpType.add
    ident = mybir.ActivationFunctionType.Identity
    expf = mybir.ActivationFunctionType.Exp

    consts = ctx.enter_context(tc.tile_pool(name="consts", bufs=1))
    xpad_pool = ctx.enter_context(tc.tile_pool(name="xpad", bufs=2))
    acc_pool = ctx.enter_context(tc.tile_pool(name="acc", bufs=2))
    h_pool = ctx.enter_context(tc.tile_pool(name="h", bufs=2))
    hkv_pool = ctx.enter_context(tc.tile_pool(name="hkv", bufs=2))
    qkv_pool = ctx.enter_context(tc.tile_pool(name="qkv", bufs=2))
    pt_pool = ctx.enter_context(tc.tile_pool(name="pt", bufs=4))
    rs_pool = ctx.enter_context(tc.tile_pool(name="rs", bufs=4))
    attn_pool = ctx.enter_context(tc.tile_pool(name="attn", bufs=2))
    out_pool = ctx.enter_context(tc.tile_pool(name="osb", bufs=3))
    psum = ctx.enter_context(tc.tile_pool(name="psum", bufs=8, space="PSUM"))

    def bank():
        return psum.tile([128, 512], f32, tag="bank", name="bank")

    # --- constants / weights: one contiguous DMA each ---
    ones = consts.tile([128, 64], bf16)
    nc.vector.memset(ones[:], 1.0)

    def load(name, shape, dt):
        t = consts.tile([128, *shape], dt, name=f"{name}_sb")
        nc.sync.dma_start(t[:], ins[name])
        return t

    xps = []
    for im in range(2):
        xp = xpad_pool.tile([128, 2, 34, 36], bf16, tag="xpad", name=f"xp{im}")
        nc.sync.dma_start(xp[:], ins[f"xp{im}"])
        xps.append(xp)
    tq_sb = load("tq", [9, 2], f32)
    dq_sb = load("dq", [len(PE_TAPS), 2, 128], bf16)
    dkv_sb = load("dkv", [9, 2, 128], bf16)
    wq_sb = load("wq", [2, 512], bf16)
    wk_sb = load("wk", [2, 512], bf16)
    wvT_sb = load("wvT", [2, 512], bf16)
    wo_sb = load("wo", [4, 256], bf16)
    qb_sb = load("qb", [4], f32)
    ob_sb = load("ob", [2], f32)

    # --- stage A: conv + pointwise per image ---
    def stage_a(im):
        xp = xps[im]

        def shift(t, kc, r0=0, rn=32, stride=1):
            dy, dx = t // 3 - 1, t % 3 - 1
            return xp[:, kc, 1 + dy + r0:1 + dy + r0 + rn:stride,
                      1 + dx:33 + dx:stride]

        # elementwise taps: GPSIMD does ts(tmp)+tt(acc+=tmp) pairs (no
        # scalar_tensor_tensor opcode on Pool); DVE chains fused ops after.
        acc = acc_pool.tile([128, 2, 32, 32], bf16, tag="acc", name=f"acc{im}")
        t0 = GP_TAPS[0]
        for kc in range(2):
            nc.gpsimd.tensor_scalar(
                acc[:, kc], shift(t0, kc), tq_sb[:, t0, kc:kc + 1], None, mult)
            for t in GP_TAPS[1:]:
                tmp = acc_pool.tile([128, 32, 32], bf16, tag="tmp",
                                    name=f"tmp{im}{kc}{t}")
                nc.gpsimd.tensor_scalar(
                    tmp[:], shift(t, kc), tq_sb[:, t, kc:kc + 1], None, mult)
                nc.gpsimd.tensor_tensor(acc[:, kc], acc[:, kc], tmp[:], add)
            for t in DVE_TAPS:
                nc.vector.scalar_tensor_tensor(
                    acc[:, kc], shift(t, kc), tq_sb[:, t, kc:kc + 1],
                    acc[:, kc], mult, add)

        # PE taps: diagonal matmuls into PSUM; combine with acc at drain.
        hq = h_pool.tile([128, 2, 32, 32], bf16, tag="hq", name=f"hq{im}")
        accf = acc.rearrange("p k h w -> p k (h w)")
        hqf = hq.rearrange("p k h w -> p k (h w)")
        for kc in range(2):
            for chk in range(2):            # 16-row chunks -> N=512
                ps = bank()
                for i, t in enumerate(PE_TAPS):
                    nc.tensor.matmul(
                        ps, dq_sb[:, i, kc, :], shift(t, kc, r0=16 * chk, rn=16),
                        start=(i == 0), stop=(i == len(PE_TAPS) - 1),
                    )
                nc.vector.tensor_tensor(
                    hqf[:, kc, chk * 512:(chk + 1) * 512], ps,
                    accf[:, kc, chk * 512:(chk + 1) * 512], add)

        # q pointwise -> [128, 4, 1024] bf16, + bias (ACT)
        q = qkv_pool.tile([128, 4, HWN], bf16, tag="q", name=f"q{im}")
        for mo in range(4):
            for chk in range(2):
                ps = bank()
                for kc in range(2):
                    nc.tensor.matmul(
                        ps, wq_sb[:, kc, mo * 128:(mo + 1) * 128],
                        hqf[:, kc, chk * 512:(chk + 1) * 512],
                        start=(kc == 0), stop=(kc == 1),
                    )
                nc.vector.tensor_scalar(
                    q[:, mo, chk * 512:(chk + 1) * 512], ps,
                    qb_sb[:, mo:mo + 1], None, add)

        # depthwise kv conv (stride 2, all taps on PE) -> hkv [128, 2, 256]
        hkv = hkv_pool.tile([128, 2, 16, 16], bf16, tag="hkv", name=f"hkv{im}")
        for kc in range(2):
            ps = bank()
            for t in range(9):
                nc.tensor.matmul(
                    ps[:, :256], dkv_sb[:, t, kc, :], shift(t, kc, stride=2),
                    start=(t == 0), stop=(t == 8),
                )
            nc.vector.tensor_copy(hkv[:, kc, :, :], ps[:, :256])

        # k pointwise -> [128, 4, 256]
        k = qkv_pool.tile([128, 4, J], bf16, tag="k", name=f"k{im}")
        hkvf = hkv.rearrange("p k h w -> p k (h w)")
        for mo in range(4):
            ps = bank()
            for kc in range(2):
                nc.tensor.matmul(
                    ps[:, :256], wk_sb[:, kc, mo * 128:(mo + 1) * 128],
                    hkvf[:, kc, :],
                    start=(kc == 0), stop=(kc == 1),
                )
            nc.scalar.copy(k[:, mo, :], ps[:, :256])

        # v transposed pointwise: vT[j, ch] -> [128, 2, 512]
        vT = qkv_pool.tile([128, 2, INNER], bf16, tag="vT", name=f"vT{im}")
        for jt in range(2):
            ps = bank()
            for kc in range(2):
                nc.tensor.matmul(
                    ps, hkvf[:, kc, jt * 128:(jt + 1) * 128], wvT_sb[:, kc, :],
                    start=(kc == 0), stop=(kc == 1),
                )
            nc.vector.tensor_copy(vT[:, jt, :], ps)

        return q, k, vT

    # --- stage B: one attention head-pair of one direction ---
    def attn_pair(q, k, vT, attn, p):
        if True:
            pt = [pt_pool.tile([128, 2, HWN], bf16, tag="pt", name=f"pt{hh_}")
                  for hh_ in range(2)]
            # scores S_T + exp; adjacent hh matmuls row-pack (K=64 at
            # partition bases 0/64 -> concurrent row groups); each score
            # matmul owns its bank (single, start+stop).
            for jt in range(2):
                for chk in range(2):
                    pss = []
                    for hh in range(2):
                        po = hh * 64
                        ps = bank()
                        nc.tensor.matmul(
                            ps,
                            k[po:po + 64, p, jt * 128:(jt + 1) * 128],
                            q[po:po + 64, p, chk * 512:(chk + 1) * 512],
                            start=True, stop=True,
                        )
                        pss.append(ps)
                    for hh in range(2):
                        nc.scalar.activation(
                            pt[hh][:, jt, chk * 512:(chk + 1) * 512],
                            pss[hh], expf)
            # sumexp: ones-matmul, M=64 output replicates the row-sum over
            # all partitions of its half; per-hh groups kept sequential.
            rs = rs_pool.tile([128, HWN], f32, tag="rs", name="rs")
            for chk in range(2):
                ps = bank()
                for hh in range(2):
                    for jt in range(2):
                        nc.tensor.matmul(
                            ps[hh * 64:hh * 64 + 64, :],
                            ones[:, :64],
                            pt[hh][:, jt, chk * 512:(chk + 1) * 512],
                            start=(jt == 0), stop=(jt == 1),
                            tile_position=(0, hh * 64),
                        )
                nc.vector.reciprocal_approx_fast(
                    rs[:, chk * 512:(chk + 1) * 512], ps)
            # PV (col-packed pairs, sequential per-hh groups) + normalization
            for chk in range(2):
                ps = bank()
                for hh in range(2):
                    for jt in range(2):
                        nc.tensor.matmul(
                            ps[hh * 64:hh * 64 + 64, :],
                            vT[:, jt, p * 128 + hh * 64:p * 128 + hh * 64 + 64],
                            pt[hh][:, jt, chk * 512:(chk + 1) * 512],
                            start=(jt == 0), stop=(jt == 1),
                            tile_position=(0, hh * 64),
                        )
                nc.vector.tensor_tensor(
                    attn[:, p, chk * 512:(chk + 1) * 512],
                    ps, rs[:, chk * 512:(chk + 1) * 512], mult)

    def outproj(attn, out_name):
        odf = outs[out_name].rearrange("(m c) h w -> c m (h w)", c=128)
        for mo in range(2):
            for chk in range(2):
                ps = bank()
                for kt in range(4):
                    nc.tensor.matmul(
                        ps, wo_sb[:, kt, mo * 128:(mo + 1) * 128],
                        attn[:, kt, chk * 512:(chk + 1) * 512],
                        start=(kt == 0), stop=(kt == 3),
                    )
                osb = out_pool.tile([128, 512], f32, tag="osb", name="osb")
                nc.scalar.activation(osb[:], ps, ident, bias=ob_sb[:, mo:mo + 1])
                nc.sync.dma_start(
                    odf[:, mo, chk * 512:(chk + 1) * 512], osb[:])

    qx, kx, vTx = stage_a(0)
    qy, ky, vTy = stage_a(1)
    attn_x = attn_pool.tile([128, 4, HWN], bf16, tag="attn", name="attn_x")
    attn_y = attn_pool.tile([128, 4, HWN], bf16, tag="attn", name="attn_y")
    for p in range(4):
        attn_pair(qx, ky, vTy, attn_x, p)
        attn_pair(qy, kx, vTx, attn_y, p)
    outproj(attn_x, "ox")
    outproj(attn_y, "oy")


def _build():
    if "nc" in _STATE:
        return _STATE["nc"]
    from contextlib import ExitStack

    import concourse.tile as tile
    from concourse import bacc, mybir

    f32 = mybir.dt.float32
    bf16 = mybir.dt.bfloat16
    nc = bacc.Bacc("TRN2", target_bir_lowering=False, debug=False)

    shapes = dict(
        xp0=((128, 2, 34, 36), bf16), xp1=((128, 2, 34, 36), bf16),
        dq=((128, len(PE_TAPS), 2, 128), bf16), dkv=((128, 9, 2, 128), bf16),
        tq=((128, 9, 2), f32),
        wq=((128, 2, 512), bf16), wk=((128, 2, 512), bf16),
        wvT=((128, 2, 512), bf16), wo=((128, 4, 256), bf16),
        qb=((128, 4), f32), ob=((128, 2), f32),
    )
    ins = {n: nc.dram_tensor(n, list(s), dt, kind="ExternalInput").ap()
           for n, (s, dt) in shapes.items()}
    outs = {n: nc.dram_tensor(n, [DIM, H, W], f32, kind="ExternalOutput").ap()
            for n in ("ox", "oy")}

    with tile.TileContext(nc) as tc, ExitStack() as ctx:
        _emit(ctx, tc, outs, ins)
    nc.compile()
    _STATE["nc"] = nc
    return nc


# ----------------------------------------------------------------------------
# public entry point
# ----------------------------------------------------------------------------

def kernel(**inputs):
    inputs = {k: np.asarray(v, dtype=np.float32) for k, v in inputs.items()}
    nc = _build()
    from concourse.bass_utils import run_bass_kernel_spmd

    w = _prep_weights(inputs)
    in_maps = []
    for b in range(NCORES):
        m = dict(w)
        m["xp0"] = _pad_image(inputs["x"][b])
        m["xp1"] = _pad_image(inputs["y"][b])
        in_maps.append(m)

    res = run_bass_kernel_spmd(nc, in_maps, list(range(NCORES))).results
    ox = np.stack([res[b]["ox"] for b in range(NCORES)])
    oy = np.stack([res[b]["oy"] for b in range(NCORES)])
    return ox, oy


# revision 12
# speedup vs baseline: 3691.9625x; 3691.9625x over previous
"""Trainium2 Bass kernel for nn_CT_Attention (conv-transformer cross-attention).

Sharding: data-parallel over batch (B=8) across 8 NeuronCores; one image pair
per core, weights replicated. No collectives.

Per-core pipeline (matmul operands bf16, accumulation fp32 in PSUM):
  host pre-pads x/y (bf16, width padded to 36 for DVE alignment) -> depthwise
  3x3 conv: taps split across PE (diagonal matmuls, PSUM accumulate), DVE and
  GPSIMD (fused multiply-accumulate), combined at drain -> pointwise convs as
  matmuls -> cross-attention with transposed scores (S_T = k^T q) so the
  softmax key-reduction runs as ones-matmuls whose M=64 output replicates the
  row-sum across partitions (normalization then needs no partition broadcast);
  V is produced directly transposed by the KV pointwise matmul -> PV with
  column-packed head pairs -> DVE normalization -> output projection + bias ->
  DMA out.  Accumulation groups sharing a PSUM bank are kept sequential
  (start=True clears has_written for the whole bank).

BN folding (host): scale into dw tap weights; bias into the q-path ACT bias
(k's bias shifts scores per-query only -> softmax invariant -> dropped; v's
bias times row-stochastic attention = constant -> folded into out-proj bias).
"""

import os
import sys

import numpy as np

for _p in ("/root/.axon_site", "/root/.axon_site/_ro/trn_rl_repo",
           "/root/.axon_site/_ro/pypackages", "/opt/trn_rl_repo", "/opt/pypackages"):
    if os.path.isdir(_p) and _p not in sys.path:
        sys.path.append(_p)

import ml_dtypes

BF16 = ml_dtypes.bfloat16

DIM = 256
HEADS = 8
DHEAD = 64
INNER = 512
SCALE = DHEAD ** -0.5
EPS = 1e-5
H = W = 32
HWN = H * W          # 1024 query positions
J = 256              # 16*16 key positions
NCORES = 8

# depthwise tap split by engine; DVE taps need dx = +-1 (t%3 != 1) so the
# bf16 2x_1P alignment (4B) holds on the width-36 padded rows.
GP_TAPS = (0, 2)
DVE_TAPS = (8, 5)
PE_TAPS = tuple(t for t in range(9) if t not in GP_TAPS and t not in DVE_TAPS)

_STATE = {}


# ----------------------------------------------------------------------------
# host-side preprocessing
# ----------------------------------------------------------------------------

def _prep_weights(inp):
    f = np.float32
    s1 = (inp["q_bn_gamma"] / np.sqrt(inp["q_bn_var"] + f(EPS))).astype(f)
    b1 = (inp["q_bn_beta"] - inp["q_bn_mean"] * s1).astype(f)
    s2 = (inp["kv_bn_gamma"] / np.sqrt(inp["kv_bn_var"] + f(EPS))).astype(f)
    b2 = (inp["kv_bn_beta"] - inp["kv_bn_mean"] * s2).astype(f)

    def diag_taps(dw_w, s, taps):
        # [128, len(taps), 2, 128] partition-major, bf16, single DMA
        d = np.zeros((128, len(taps), 2, 128), f)
        idx = np.arange(128)
        for i, t in enumerate(taps):
            wt = dw_w[:, 0, t // 3, t % 3] * s          # [256]
            for kc in range(2):
                d[idx, i, kc, idx] = wt[kc * 128:(kc + 1) * 128]
        return d.astype(BF16)

    def tapw(dw_w, s):
        # per-partition tap scalars [128, 9, 2] fp32
        w = (dw_w[:, 0].reshape(256, 9) * s[:, None]).astype(f)    # [256, 9]
        return np.ascontiguousarray(
            w.reshape(2, 128, 9).transpose(1, 2, 0))               # [128, 9, 2]

    dq = diag_taps(inp["q_dw_w"], s1, PE_TAPS)
    dkv = diag_taps(inp["kv_dw_w"], s2, tuple(range(9)))
    tq = tapw(inp["q_dw_w"], s1)

    qpw = inp["q_pw_w"].astype(f)                        # [512, 256]
    wq = np.ascontiguousarray(
        (f(SCALE) * qpw).T.reshape(2, 128, 512).transpose(1, 0, 2)).astype(BF16)
    qb = (f(SCALE) * (qpw @ b1)).reshape(4, 128).T.copy()  # [128, 4] fp32

    kvpw = inp["kv_pw_w"].astype(f)                      # [1024, 256]
    wk = np.ascontiguousarray(
        kvpw[:512].T.reshape(2, 128, 512).transpose(1, 0, 2)).astype(BF16)
    wvT = np.ascontiguousarray(
        kvpw[512:].T.reshape(2, 128, 512).transpose(1, 0, 2)).astype(BF16)
    vb = kvpw[512:] @ b2                                 # [512]

    ow = inp["out_w"].astype(f)                          # [256, 512]
    wo = np.ascontiguousarray(
        ow.T.reshape(4, 128, 256).transpose(1, 0, 2)).astype(BF16)
    ob = (inp["out_b"].astype(f) + ow @ vb).reshape(2, 128).T.copy()  # [128, 2]

    return dict(dq=dq, dkv=dkv, tq=tq, wq=wq, wk=wk, wvT=wvT, wo=wo,
                qb=qb, ob=ob)


def _pad_image(img):
    # [256, 32, 32] -> [128, 2, 34, 36] zero-padded bf16, partition-major
    out = np.zeros((128, 2, 34, 36), BF16)
    out[:, :, 1:33, 1:33] = img.reshape(2, 128, 32, 32).transpose(1, 0, 2, 3)
    return out


# ----------------------------------------------------------------------------
# device program
# ----------------------------------------------------------------------------

def _emit(ctx, tc, outs, ins, n_iters=1):
    import concourse.bass as bass  # noqa: F401
    from concourse import mybir

    nc = tc.nc
    f32 = mybir.dt.float32
    bf16 = mybir.dt.bfloat16
    mult = mybir.AluOpType.mult
    add = mybir.AluOpType.add
    ident = mybir.ActivationFunctionType.Identity
    expf = mybir.ActivationFunctionType.Exp

    consts = ctx.enter_context(tc.tile_pool(name="consts", bufs=1))
    xpad_pool = ctx.enter_context(tc.tile_pool(name="xpad", bufs=2))
    acc_pool = ctx.enter_context(tc.tile_pool(name="acc", bufs=2))
    h_pool = ctx.enter_context(tc.tile_pool(name="h", bufs=2))
    hkv_pool = ctx.enter_context(tc.tile_pool(name="hkv", bufs=2))
    qkv_pool = ctx.enter_context(tc.tile_pool(name="qkv", bufs=2))
    pt_pool = ctx.enter_context(tc.tile_pool(name="pt", bufs=4))
    rs_pool = ctx.enter_context(tc.tile_pool(name="rs", bufs=4))
    attn_pool = ctx.enter_context(tc.tile_pool(name="attn", bufs=2))
    out_pool = ctx.enter_context(tc.tile_pool(name="osb", bufs=3))
    psum = ctx.enter_context(tc.tile_pool(name="psum", bufs=8, space="PSUM"))

    def bank():
        return psum.tile([128, 512], f32, tag="bank", name="bank")

    # --- constants / weights: one contiguous DMA each ---
    ones = consts.tile([128, 64], bf16)
    nc.vector.memset(ones[:], 1.0)

    def load(name, shape, dt):
        t = consts.tile([128, *shape], dt, name=f"{name}_sb")
        nc.sync.dma_start(t[:], ins[name])
        return t

    sfx = [""]
    tq_sb = load("tq", [9, 2], f32)
    dq_sb = load("dq", [len(PE_TAPS), 2, 128], bf16)
    dkv_sb = load("dkv", [9, 2, 128], bf16)
    wq_sb = load("wq", [2, 512], bf16)
    wk_sb = load("wk", [2, 512], bf16)
    wvT_sb = load("wvT", [2, 512], bf16)
    wo_sb = load("wo", [4, 256], bf16)
    qb_sb = load("qb", [4], f32)
    ob_sb = load("ob", [2], f32)

    # --- stage A: conv + pointwise per image ---
    def stage_a(im):
        xp = xpad_pool.tile([128, 2, 34, 36], bf16, tag="xpad",
                            name=f"xp{im}{sfx[0]}")
        nc.sync.dma_start(xp[:], ins[f"xp{im}"])

        def shift(t, kc, r0=0, rn=32, stride=1):
            dy, dx = t // 3 - 1, t % 3 - 1
            return xp[:, kc, 1 + dy + r0:1 + dy + r0 + rn:stride,
                      1 + dx:33 + dx:stride]

        # elementwise taps: GPSIMD does ts(tmp)+tt(acc+=tmp) pairs (no
        # scalar_tensor_tensor opcode on Pool); DVE chains fused ops after.
        acc = acc_pool.tile([128, 2, 32, 32], bf16, tag="acc", name=f"acc{im}{sfx[0]}")
        t0 = GP_TAPS[0]
        for kc in range(2):
            nc.gpsimd.tensor_scalar(
                acc[:, kc], shift(t0, kc), tq_sb[:, t0, kc:kc + 1], None, mult)
            for t in GP_TAPS[1:]:
                tmp = acc_pool.tile([128, 32, 32], bf16, tag="tmp",
                                    name=f"tmp{im}{kc}{t}{sfx[0]}")
                nc.gpsimd.tensor_scalar(
                    tmp[:], shift(t, kc), tq_sb[:, t, kc:kc + 1], None, mult)
                nc.gpsimd.tensor_tensor(acc[:, kc], acc[:, kc], tmp[:], add)
            for t in DVE_TAPS:
                nc.vector.scalar_tensor_tensor(
                    acc[:, kc], shift(t, kc), tq_sb[:, t, kc:kc + 1],
                    acc[:, kc], mult, add)

        # PE taps: diagonal matmuls into PSUM; combine with acc at drain.
        hq = h_pool.tile([128, 2, 32, 32], bf16, tag="hq", name=f"hq{im}{sfx[0]}")
        accf = acc.rearrange("p k h w -> p k (h w)")
        hqf = hq.rearrange("p k h w -> p k (h w)")
        for kc in range(2):
            for chk in range(2):            # 16-row chunks -> N=512
                ps = bank()
                for i, t in enumerate(PE_TAPS):
                    nc.tensor.matmul(
                        ps, dq_sb[:, i, kc, :], shift(t, kc, r0=16 * chk, rn=16),
                        start=(i == 0), stop=(i == len(PE_TAPS) - 1),
                    )
                nc.vector.tensor_tensor(
                    hqf[:, kc, chk * 512:(chk + 1) * 512], ps,
                    accf[:, kc, chk * 512:(chk + 1) * 512], add)

        # q pointwise -> [128, 4, 1024] bf16, + bias (ACT)
        q = qkv_pool.tile([128, 4, HWN], bf16, tag="q", name=f"q{im}{sfx[0]}")
        for mo in range(4):
            for chk in range(2):
                ps = bank()
                for kc in range(2):
                    nc.tensor.matmul(
                        ps, wq_sb[:, kc, mo * 128:(mo + 1) * 128],
                        hqf[:, kc, chk * 512:(chk + 1) * 512],
                        start=(kc == 0), stop=(kc == 1),
                    )
                nc.vector.tensor_scalar(
                    q[:, mo, chk * 512:(chk + 1) * 512], ps,
                    qb_sb[:, mo:mo + 1], None, add)

        # depthwise kv conv (stride 2, all taps on PE) -> hkv [128, 2, 256]
        hkv = hkv_pool.tile([128, 2, 16, 16], bf16, tag="hkv", name=f"hkv{im}{sfx[0]}")
        for kc in range(2):
            ps = bank()
            for t in range(9):
                nc.tensor.matmul(
                    ps[:, :256], dkv_sb[:, t, kc, :], shift(t, kc, stride=2),
                    start=(t == 0), stop=(t == 8),
                )
            nc.vector.tensor_copy(hkv[:, kc, :, :], ps[:, :256])

        # k pointwise -> [128, 4, 256]
        k = qkv_pool.tile([128, 4, J], bf16, tag="k", name=f"k{im}{sfx[0]}")
        hkvf = hkv.rearrange("p k h w -> p k (h w)")
        for mo in range(4):
            ps = bank()
            for kc in range(2):
                nc.tensor.matmul(
                    ps[:, :256], wk_sb[:, kc, mo * 128:(mo + 1) * 128],
                    hkvf[:, kc, :],
                    start=(kc == 0), stop=(kc == 1),
                )
            nc.scalar.copy(k[:, mo, :], ps[:, :256])

        # v transposed pointwise: vT[j, ch] -> [128, 2, 512]
        vT = qkv_pool.tile([128, 2, INNER], bf16, tag="vT", name=f"vT{im}{sfx[0]}")
        for jt in range(2):
            ps = bank()
            for kc in range(2):
                nc.tensor.matmul(
                    ps, hkvf[:, kc, jt * 128:(jt + 1) * 128], wvT_sb[:, kc, :],
                    start=(kc == 0), stop=(kc == 1),
                )
            nc.vector.tensor_copy(vT[:, jt, :], ps)

        return q, k, vT

    # --- stage B: one attention head-pair of one direction ---
    def attn_pair(q, k, vT, attn, p):
        if True:
            pt = [pt_pool.tile([128, 2, HWN], bf16, tag="pt", name=f"pt{hh_}")
                  for hh_ in range(2)]
            # scores S_T + exp; adjacent hh matmuls row-pack (K=64 at
            # partition bases 0/64 -> concurrent row groups); each score
            # matmul owns its bank (single, start+stop).
            for jt in range(2):
                for chk in range(2):
                    pss = []
                    for hh in range(2):
                        po = hh * 64
                        ps = bank()
                        nc.tensor.matmul(
                            ps,
                            k[po:po + 64, p, jt * 128:(jt + 1) * 128],
                            q[po:po + 64, p, chk * 512:(chk + 1) * 512],
                            start=True, stop=True,
                        )
                        pss.append(ps)
                    for hh in range(2):
                        nc.scalar.activation(
                            pt[hh][:, jt, chk * 512:(chk + 1) * 512],
                            pss[hh], expf)
            # sumexp: ones-matmul, M=64 output replicates the row-sum over
            # all partitions of its half; per-hh groups kept sequential.
            rs = rs_pool.tile([128, HWN], f32, tag="rs", name="rs")
            for chk in range(2):
                ps = bank()
                for hh in range(2):
                    for jt in range(2):
                        nc.tensor.matmul(
                            ps[hh * 64:hh * 64 + 64, :],
                            ones[:, :64],
                            pt[hh][:, jt, chk * 512:(chk + 1) * 512],
                            start=(jt == 0), stop=(jt == 1),
                            tile_position=(0, hh * 64),
                        )
                nc.vector.reciprocal_approx_fast(
                    rs[:, chk * 512:(chk + 1) * 512], ps)
            # PV (col-packed pairs, sequential per-hh groups) + normalization
            for chk in range(2):
                ps = bank()
                for hh in range(2):
                    for jt in range(2):
                        nc.tensor.matmul(
                            ps[hh * 64:hh * 64 + 64, :],
                            vT[:, jt, p * 128 + hh * 64:p * 128 + hh * 64 + 64],
                            pt[hh][:, jt, chk * 512:(chk + 1) * 512],
                            start=(jt == 0), stop=(jt == 1),
                            tile_position=(0, hh * 64),
                        )
                nc.vector.tensor_tensor(
                    attn[:, p, chk * 512:(chk + 1) * 512],
                    ps, rs[:, chk * 512:(chk + 1) * 512], mult)

    def outproj(attn, out_name):
        odf = outs[out_name].rearrange("(m c) h w -> c m (h w)", c=128)
        for mo in range(2):
            for chk in range(2):
                ps = bank()
                for kt in range(4):
                    nc.tensor.matmul(
                        ps, wo_sb[:, kt, mo * 128:(mo + 1) * 128],
                        attn[:, kt, chk * 512:(chk + 1) * 512],
                        start=(kt == 0), stop=(kt == 3),
                    )
                osb = out_pool.tile([128, 512], f32, tag="osb", name="osb")
                nc.scalar.activation(osb[:], ps, ident, bias=ob_sb[:, mo:mo + 1])
                nc.sync.dma_start(
                    odf[:, mo, chk * 512:(chk + 1) * 512], osb[:])

    for _it in range(n_iters):
        sfx[0] = f"_{_it}"
        qx, kx, vTx = stage_a(0)
        qy, ky, vTy = stage_a(1)
        attn_x = attn_pool.tile([128, 4, HWN], bf16, tag="attn",
                                name=f"attn_x{_it}")
        attn_y = attn_pool.tile([128, 4, HWN], bf16, tag="attn",
                                name=f"attn_y{_it}")
        for p in range(4):
            attn_pair(qx, ky, vTy, attn_x, p)
            attn_pair(qy, kx, vTx, attn_y, p)
        outproj(attn_x, "ox")
        outproj(attn_y, "oy")


def _build(n_iters=1):
    if ("nc", n_iters) in _STATE:
        return _STATE[("nc", n_iters)]
    from contextlib import ExitStack

    import concourse.tile as tile
    from concourse import bacc, mybir

    f32 = mybir.dt.float32
    bf16 = mybir.dt.bfloat16
    nc = bacc.Bacc("TRN2", target_bir_lowering=False, debug=False)

    shapes = dict(
        xp0=((128, 2, 34, 36), bf16), xp1=((128, 2, 34, 36), bf16),
        dq=((128, len(PE_TAPS), 2, 128), bf16), dkv=((128, 9, 2, 128), bf16),
        tq=((128, 9, 2), f32),
        wq=((128, 2, 512), bf16), wk=((128, 2, 512), bf16),
        wvT=((128, 2, 512), bf16), wo=((128, 4, 256), bf16),
        qb=((128, 4), f32), ob=((128, 2), f32),
    )
    ins = {n: nc.dram_tensor(n, list(s), dt, kind="ExternalInput").ap()
           for n, (s, dt) in shapes.items()}
    outs = {n: nc.dram_tensor(n, [DIM, H, W], f32, kind="ExternalOutput").ap()
            for n in ("ox", "oy")}

    with tile.TileContext(nc) as tc, ExitStack() as ctx:
        _emit(ctx, tc, outs, ins, n_iters=n_iters)
    nc.compile()
    _STATE[("nc", n_iters)] = nc
    return nc


# ----------------------------------------------------------------------------
# public entry point
# ----------------------------------------------------------------------------

def kernel(**inputs):
    inputs = {k: np.asarray(v, dtype=np.float32) for k, v in inputs.items()}
    nc = _build(1)
    from concourse.bass_utils import run_bass_kernel_spmd

    w = _prep_weights(inputs)
    in_maps = []
    for b in range(NCORES):
        m = dict(w)
        m["xp0"] = _pad_image(inputs["x"][b])
        m["xp1"] = _pad_image(inputs["y"][b])
        in_maps.append(m)

    res = run_bass_kernel_spmd(nc, in_maps, list(range(NCORES))).results
    ox = np.stack([res[b]["ox"] for b in range(NCORES)])
    oy = np.stack([res[b]["oy"] for b in range(NCORES)])
    return ox, oy


# revision 13
# speedup vs baseline: 11059.5930x; 2.9956x over previous
"""Trainium2 Bass kernel for nn_CT_Attention (conv-transformer cross-attention).

Sharding: data-parallel over batch (B=8) across 8 NeuronCores; one image pair
per core, weights replicated. No collectives.

Per-core pipeline (matmul operands bf16, accumulation fp32 in PSUM):
  host pre-pads x/y (bf16, width padded to 36 for DVE alignment) -> depthwise
  3x3 conv: taps split across PE (diagonal matmuls, PSUM accumulate), DVE and
  GPSIMD (fused multiply-accumulate), combined at drain -> pointwise convs as
  matmuls -> cross-attention with transposed scores (S_T = k^T q) so the
  softmax key-reduction runs as ones-matmuls whose M=64 output replicates the
  row-sum across partitions (normalization then needs no partition broadcast);
  V is produced directly transposed by the KV pointwise matmul -> PV with
  column-packed head pairs -> DVE normalization -> output projection + bias ->
  DMA out.  Accumulation groups sharing a PSUM bank are kept sequential
  (start=True clears has_written for the whole bank).

BN folding (host): scale into dw tap weights; bias into the q-path ACT bias
(k's bias shifts scores per-query only -> softmax invariant -> dropped; v's
bias times row-stochastic attention = constant -> folded into out-proj bias).
"""

import os
import sys

import numpy as np

for _p in ("/root/.axon_site", "/root/.axon_site/_ro/trn_rl_repo",
           "/root/.axon_site/_ro/pypackages", "/opt/trn_rl_repo", "/opt/pypackages"):
    if os.path.isdir(_p) and _p not in sys.path:
        sys.path.append(_p)

import ml_dtypes

BF16 = ml_dtypes.bfloat16

DIM = 256
HEADS = 8
DHEAD = 64
INNER = 512
SCALE = DHEAD ** -0.5
EPS = 1e-5
H = W = 32
HWN = H * W          # 1024 query positions
J = 256              # 16*16 key positions
NCORES = 8

# depthwise tap split by engine; DVE taps need dx = +-1 (t%3 != 1) so the
# bf16 2x_1P alignment (4B) holds on the width-36 padded rows.
GP_TAPS = (0, 2)
DVE_TAPS = (8, 5)
PE_TAPS = tuple(t for t in range(9) if t not in GP_TAPS and t not in DVE_TAPS)

_STATE = {}


# ----------------------------------------------------------------------------
# host-side preprocessing
# ----------------------------------------------------------------------------

def _prep_weights(inp):
    f = np.float32
    s1 = (inp["q_bn_gamma"] / np.sqrt(inp["q_bn_var"] + f(EPS))).astype(f)
    b1 = (inp["q_bn_beta"] - inp["q_bn_mean"] * s1).astype(f)
    s2 = (inp["kv_bn_gamma"] / np.sqrt(inp["kv_bn_var"] + f(EPS))).astype(f)
    b2 = (inp["kv_bn_beta"] - inp["kv_bn_mean"] * s2).astype(f)

    def diag_taps(dw_w, s, taps):
        # [128, len(taps), 2, 128] partition-major, bf16, single DMA
        d = np.zeros((128, len(taps), 2, 128), f)
        idx = np.arange(128)
        for i, t in enumerate(taps):
            wt = dw_w[:, 0, t // 3, t % 3] * s          # [256]
            for kc in range(2):
                d[idx, i, kc, idx] = wt[kc * 128:(kc + 1) * 128]
        return d.astype(BF16)

    def tapw(dw_w, s):
        # per-partition tap scalars [128, 9, 2] fp32
        w = (dw_w[:, 0].reshape(256, 9) * s[:, None]).astype(f)    # [256, 9]
        return np.ascontiguousarray(
            w.reshape(2, 128, 9).transpose(1, 2, 0))               # [128, 9, 2]

    dq = diag_taps(inp["q_dw_w"], s1, PE_TAPS)
    dkv = diag_taps(inp["kv_dw_w"], s2, tuple(range(9)))
    tq = tapw(inp["q_dw_w"], s1)

    qpw = inp["q_pw_w"].astype(f)                        # [512, 256]
    wq = np.ascontiguousarray(
        (f(SCALE) * qpw).T.reshape(2, 128, 512).transpose(1, 0, 2)).astype(BF16)
    qb = (f(SCALE) * (qpw @ b1)).reshape(4, 128).T.copy()  # [128, 4] fp32

    kvpw = inp["kv_pw_w"].astype(f)                      # [1024, 256]
    wk = np.ascontiguousarray(
        kvpw[:512].T.reshape(2, 128, 512).transpose(1, 0, 2)).astype(BF16)
    wvT = np.ascontiguousarray(
        kvpw[512:].T.reshape(2, 128, 512).transpose(1, 0, 2)).astype(BF16)
    vb = kvpw[512:] @ b2                                 # [512]

    ow = inp["out_w"].astype(f)                          # [256, 512]
    wo = np.ascontiguousarray(
        ow.T.reshape(4, 128, 256).transpose(1, 0, 2)).astype(BF16)
    ob = (inp["out_b"].astype(f) + ow @ vb).reshape(2, 128).T.copy()  # [128, 2]

    return dict(dq=dq, dkv=dkv, tq=tq, wq=wq, wk=wk, wvT=wvT, wo=wo,
                qb=qb, ob=ob)


def _pad_image(img):
    # [256, 32, 32] -> [128, 2, 34, 36] zero-padded bf16, partition-major
    out = np.zeros((128, 2, 34, 36), BF16)
    out[:, :, 1:33, 1:33] = img.reshape(2, 128, 32, 32).transpose(1, 0, 2, 3)
    return out


# ----------------------------------------------------------------------------
# device program
# ----------------------------------------------------------------------------

def _emit(ctx, tc, outs, ins, n_iters=1):
    import concourse.bass as bass  # noqa: F401
    from concourse import mybir

    nc = tc.nc
    f32 = mybir.dt.float32
    bf16 = mybir.dt.bfloat16
    mult = mybir.AluOpType.mult
    add = mybir.AluOpType.add
    ident = mybir.ActivationFunctionType.Identity
    expf = mybir.ActivationFunctionType.Exp

    consts = ctx.enter_context(tc.tile_pool(name="consts", bufs=1))
    xpad_pool = ctx.enter_context(tc.tile_pool(name="xpad", bufs=4))
    acc_pool = ctx.enter_context(tc.tile_pool(name="acc", bufs=4))
    h_pool = ctx.enter_context(tc.tile_pool(name="h", bufs=4))
    hkv_pool = ctx.enter_context(tc.tile_pool(name="hkv", bufs=4))
    qkv_pool = ctx.enter_context(tc.tile_pool(name="qkv", bufs=2))
    pt_pool = ctx.enter_context(tc.tile_pool(name="pt", bufs=4))
    rs_pool = ctx.enter_context(tc.tile_pool(name="rs", bufs=4))
    attn_pool = ctx.enter_context(tc.tile_pool(name="attn", bufs=2))
    out_pool = ctx.enter_context(tc.tile_pool(name="osb", bufs=4))
    psum = ctx.enter_context(tc.tile_pool(name="psum", bufs=8, space="PSUM"))

    def bank():
        return psum.tile([128, 512], f32, tag="bank", name="bank")

    # --- constants / weights: one contiguous DMA each ---
    ones = consts.tile([128, 64], bf16)
    nc.vector.memset(ones[:], 1.0)

    def load(name, shape, dt):
        t = consts.tile([128, *shape], dt, name=f"{name}_sb")
        nc.sync.dma_start(t[:], ins[name])
        return t

    sfx = [""]
    tq_sb = load("tq", [9, 2], f32)
    dq_sb = load("dq", [len(PE_TAPS), 2, 128], bf16)
    dkv_sb = load("dkv", [9, 2, 128], bf16)
    wq_sb = load("wq", [2, 512], bf16)
    wk_sb = load("wk", [2, 512], bf16)
    wvT_sb = load("wvT", [2, 512], bf16)
    wo_sb = load("wo", [4, 256], bf16)
    qb_sb = load("qb", [4], f32)
    ob_sb = load("ob", [2], f32)

    # --- stage A: conv + pointwise per image ---
    def stage_a(im):
        xp = xpad_pool.tile([128, 2, 34, 36], bf16, tag="xpad",
                            name=f"xp{im}{sfx[0]}")
        nc.sync.dma_start(xp[:], ins[f"xp{im}"])

        def shift(t, kc, r0=0, rn=32, stride=1):
            dy, dx = t // 3 - 1, t % 3 - 1
            return xp[:, kc, 1 + dy + r0:1 + dy + r0 + rn:stride,
                      1 + dx:33 + dx:stride]

        # elementwise taps: GPSIMD does ts(tmp)+tt(acc+=tmp) pairs (no
        # scalar_tensor_tensor opcode on Pool); DVE chains fused ops after.
        acc = acc_pool.tile([128, 2, 32, 32], bf16, tag="acc", name=f"acc{im}{sfx[0]}")
        t0 = GP_TAPS[0]
        for kc in range(2):
            nc.gpsimd.tensor_scalar(
                acc[:, kc], shift(t0, kc), tq_sb[:, t0, kc:kc + 1], None, mult)
            for t in GP_TAPS[1:]:
                tmp = acc_pool.tile([128, 32, 32], bf16, tag="tmp",
                                    name=f"tmp{im}{kc}{t}{sfx[0]}")
                nc.gpsimd.tensor_scalar(
                    tmp[:], shift(t, kc), tq_sb[:, t, kc:kc + 1], None, mult)
                nc.gpsimd.tensor_tensor(acc[:, kc], acc[:, kc], tmp[:], add)
            for t in DVE_TAPS:
                nc.vector.scalar_tensor_tensor(
                    acc[:, kc], shift(t, kc), tq_sb[:, t, kc:kc + 1],
                    acc[:, kc], mult, add)

        # PE taps: diagonal matmuls into PSUM; combine with acc at drain.
        hq = h_pool.tile([128, 2, 32, 32], bf16, tag="hq", name=f"hq{im}{sfx[0]}")
        accf = acc.rearrange("p k h w -> p k (h w)")
        hqf = hq.rearrange("p k h w -> p k (h w)")
        for kc in range(2):
            for chk in range(2):            # 16-row chunks -> N=512
                ps = bank()
                for i, t in enumerate(PE_TAPS):
                    nc.tensor.matmul(
                        ps, dq_sb[:, i, kc, :], shift(t, kc, r0=16 * chk, rn=16),
                        start=(i == 0), stop=(i == len(PE_TAPS) - 1),
                    )
                nc.vector.tensor_tensor(
                    hqf[:, kc, chk * 512:(chk + 1) * 512], ps,
                    accf[:, kc, chk * 512:(chk + 1) * 512], add)

        # q pointwise -> [128, 4, 1024] bf16, + bias (ACT)
        q = qkv_pool.tile([128, 4, HWN], bf16, tag="q", name=f"q{im}{sfx[0]}")
        for mo in range(4):
            for chk in range(2):
                ps = bank()
                for kc in range(2):
                    nc.tensor.matmul(
                        ps, wq_sb[:, kc, mo * 128:(mo + 1) * 128],
                        hqf[:, kc, chk * 512:(chk + 1) * 512],
                        start=(kc == 0), stop=(kc == 1),
                    )
                nc.vector.tensor_scalar(
                    q[:, mo, chk * 512:(chk + 1) * 512], ps,
                    qb_sb[:, mo:mo + 1], None, add)

        # depthwise kv conv (stride 2, all taps on PE) -> hkv [128, 2, 256]
        hkv = hkv_pool.tile([128, 2, 16, 16], bf16, tag="hkv", name=f"hkv{im}{sfx[0]}")
        for kc in range(2):
            ps = bank()
            for t in range(9):
                nc.tensor.matmul(
                    ps[:, :256], dkv_sb[:, t, kc, :], shift(t, kc, stride=2),
                    start=(t == 0), stop=(t == 8),
                )
            nc.vector.tensor_copy(hkv[:, kc, :, :], ps[:, :256])

        # k pointwise -> [128, 4, 256]
        k = qkv_pool.tile([128, 4, J], bf16, tag="k", name=f"k{im}{sfx[0]}")
        hkvf = hkv.rearrange("p k h w -> p k (h w)")
        for mo in range(4):
            ps = bank()
            for kc in range(2):
                nc.tensor.matmul(
                    ps[:, :256], wk_sb[:, kc, mo * 128:(mo + 1) * 128],
                    hkvf[:, kc, :],
                    start=(kc == 0), stop=(kc == 1),
                )
            nc.scalar.copy(k[:, mo, :], ps[:, :256])

        # v transposed pointwise: vT[j, ch] -> [128, 2, 512]
        vT = qkv_pool.tile([128, 2, INNER], bf16, tag="vT", name=f"vT{im}{sfx[0]}")
        for jt in range(2):
            ps = bank()
            for kc in range(2):
                nc.tensor.matmul(
                    ps, hkvf[:, kc, jt * 128:(jt + 1) * 128], wvT_sb[:, kc, :],
                    start=(kc == 0), stop=(kc == 1),
                )
            nc.vector.tensor_copy(vT[:, jt, :], ps)

        return q, k, vT

    # --- stage B: one attention head-pair of one direction ---
    def attn_pair(q, k, vT, attn, p):
        if True:
            pt = [pt_pool.tile([128, 2, HWN], bf16, tag="pt", name=f"pt{hh_}")
                  for hh_ in range(2)]
            # scores S_T + exp; adjacent hh matmuls row-pack (K=64 at
            # partition bases 0/64 -> concurrent row groups); each score
            # matmul owns its bank (single, start+stop).
            for jt in range(2):
                for chk in range(2):
                    pss = []
                    for hh in range(2):
                        po = hh * 64
                        ps = bank()
                        nc.tensor.matmul(
                            ps,
                            k[po:po + 64, p, jt * 128:(jt + 1) * 128],
                            q[po:po + 64, p, chk * 512:(chk + 1) * 512],
                            start=True, stop=True,
                        )
                        pss.append(ps)
                    for hh in range(2):
                        nc.scalar.activation(
                            pt[hh][:, jt, chk * 512:(chk + 1) * 512],
                            pss[hh], expf)
            # sumexp: ones-matmul, M=64 output replicates the row-sum over
            # all partitions of its half; per-hh groups kept sequential.
            rs = rs_pool.tile([128, HWN], f32, tag="rs", name="rs")
            for chk in range(2):
                ps = bank()
                for hh in range(2):
                    for jt in range(2):
                        nc.tensor.matmul(
                            ps[hh * 64:hh * 64 + 64, :],
                            ones[:, :64],
                            pt[hh][:, jt, chk * 512:(chk + 1) * 512],
                            start=(jt == 0), stop=(jt == 1),
                            tile_position=(0, hh * 64),
                        )
                nc.vector.reciprocal_approx_fast(
                    rs[:, chk * 512:(chk + 1) * 512], ps)
            # PV (col-packed pairs, sequential per-hh groups) + normalization
            for chk in range(2):
                ps = bank()
                for hh in range(2):
                    for jt in range(2):
                        nc.tensor.matmul(
                            ps[hh * 64:hh * 64 + 64, :],
                            vT[:, jt, p * 128 + hh * 64:p * 128 + hh * 64 + 64],
                            pt[hh][:, jt, chk * 512:(chk + 1) * 512],
                            start=(jt == 0), stop=(jt == 1),
                            tile_position=(0, hh * 64),
                        )
                nc.vector.tensor_tensor(
                    attn[:, p, chk * 512:(chk + 1) * 512],
                    ps, rs[:, chk * 512:(chk + 1) * 512], mult)

    def outproj(attn, out_name):
        odf = outs[out_name].rearrange("(m c) h w -> c m (h w)", c=128)
        for mo in range(2):
            for chk in range(2):
                ps = bank()
                for kt in range(4):
                    nc.tensor.matmul(
                        ps, wo_sb[:, kt, mo * 128:(mo + 1) * 128],
                        attn[:, kt, chk * 512:(chk + 1) * 512],
                        start=(kt == 0), stop=(kt == 3),
                    )
                osb = out_pool.tile([128, 512], f32, tag="osb", name="osb")
                nc.scalar.activation(osb[:], ps, ident, bias=ob_sb[:, mo:mo + 1])
                nc.sync.dma_start(
                    odf[:, mo, chk * 512:(chk + 1) * 512], osb[:])

    for _it in range(n_iters):
        sfx[0] = f"_{_it}"
        qx, kx, vTx = stage_a(0)
        qy, ky, vTy = stage_a(1)
        attn_x = attn_pool.tile([128, 4, HWN], bf16, tag="attn",
                                name=f"attn_x{_it}")
        attn_y = attn_pool.tile([128, 4, HWN], bf16, tag="attn",
                                name=f"attn_y{_it}")
        for p in range(4):
            attn_pair(qx, ky, vTy, attn_x, p)
            attn_pair(qy, kx, vTx, attn_y, p)
        outproj(attn_x, "ox")
        outproj(attn_y, "oy")


def _build(n_iters=1):
    if ("nc", n_iters) in _STATE:
        return _STATE[("nc", n_iters)]
    from contextlib import ExitStack

    import concourse.tile as tile
    from concourse import bacc, mybir

    f32 = mybir.dt.float32
    bf16 = mybir.dt.bfloat16
    nc = bacc.Bacc("TRN2", target_bir_lowering=False, debug=False)

    shapes = dict(
        xp0=((128, 2, 34, 36), bf16), xp1=((128, 2, 34, 36), bf16),
        dq=((128, len(PE_TAPS), 2, 128), bf16), dkv=((128, 9, 2, 128), bf16),
        tq=((128, 9, 2), f32),
        wq=((128, 2, 512), bf16), wk=((128, 2, 512), bf16),
        wvT=((128, 2, 512), bf16), wo=((128, 4, 256), bf16),
        qb=((128, 4), f32), ob=((128, 2), f32),
    )
    ins = {n: nc.dram_tensor(n, list(s), dt, kind="ExternalInput").ap()
           for n, (s, dt) in shapes.items()}
    outs = {n: nc.dram_tensor(n, [DIM, H, W], f32, kind="ExternalOutput").ap()
            for n in ("ox", "oy")}

    with tile.TileContext(nc) as tc, ExitStack() as ctx:
        _emit(ctx, tc, outs, ins, n_iters=n_iters)
    nc.compile()
    _STATE[("nc", n_iters)] = nc
    return nc


# ----------------------------------------------------------------------------
# public entry point
# ----------------------------------------------------------------------------

def kernel(**inputs):
    inputs = {k: np.asarray(v, dtype=np.float32) for k, v in inputs.items()}
    nc = _build(1)
    from concourse.bass_utils import run_bass_kernel_spmd

    w = _prep_weights(inputs)
    in_maps = []
    for b in range(NCORES):
        m = dict(w)
        m["xp0"] = _pad_image(inputs["x"][b])
        m["xp1"] = _pad_image(inputs["y"][b])
        in_maps.append(m)

    res = run_bass_kernel_spmd(nc, in_maps, list(range(NCORES))).results
    ox = np.stack([res[b]["ox"] for b in range(NCORES)])
    oy = np.stack([res[b]["oy"] for b in range(NCORES)])
    return ox, oy
